# revision 17
# baseline (speedup 1.0000x reference)
"""Trainium2 Bass kernel for nn_DSDModules_57681410785615 (sparse_attention).

Strategy (expert-parallel over the group axis G=8, one group per NeuronCore):
  - Each core runs its group's 4-layer 3x3 conv stack as shifted matmuls in
    fp8e4 with DoubleRow perf mode (2 contraction rows per partition):
    conv1 pairs input-channel slices; conv2..4 pair adjacent taps using a
    second, 1-pixel-shifted fp8 copy of the activations (written by the
    Scalar engine, which also fuses bias + leaky-relu + 1/64 weight-descale).
  - conv4 drain fuses exp() for the logit rows; cross-group softmax via an
    8-core AllReduce of exp(logits), overlapped with the offset->bilinear
    weight map computation on DVE.
  - The bilinear warp is a 9-point spatially-varying stencil: per-pixel
    stencil weights fold the K=6 taps, bilinear fractions and softmax attn
    into 9 maps, reduced over K on the TensorEngine and broadcast across
    partitions via a DRAM round-trip DMA (spread over queues).
  - The group mask is pre-multiplied into the warp source image on the host.
  - Each core DMAs out its masked partial [256, 64*64] bf16; the host sums
    the 8 partials (the reference's sum over groups) in fp32.

Self-contained: hardcodes all shapes; no file reads.
"""
import sys
import contextlib

for _p in ('/opt/trn_rl_repo', '/opt/trn_rl_repo/concourse'):
    if _p not in sys.path:
        sys.path.insert(0, _p)

import numpy as np
import ml_dtypes

import concourse.bass as bass
import concourse.mybir as mybir
import concourse.tile as tile
from concourse import bacc
from concourse.bass_utils import run_bass_kernel_spmd

BF16 = ml_dtypes.bfloat16
FP8 = ml_dtypes.float8_e4m3
F32 = mybir.dt.float32
BF = mybir.dt.bfloat16
F8 = mybir.dt.float8e4
I32 = mybir.dt.int32
PM = mybir.MatmulPerfMode

G, K, C_IN, C_FEAT, H, W, B = 8, 6, 512, 256, 64, 64, 1
HW = H * W                  # 4096
PADW = 66                   # padded conv row width
NPAD = PADW * PADW          # 4356 padded conv pixels
GUARD = 66                  # flat warp-image guard elements (even)
NFLAT = GUARD + HW + GUARD  # 4228
WS = 64.0                   # fp8 weight scale
AluOp = mybir.AluOpType
ActFn = mybir.ActivationFunctionType

# conv4 output channel permutation: [logit_k (6), offx_k (6), offy_k (6)]
PERM4 = [12 + k for k in range(K)] + [2 * k for k in range(K)] + [2 * k + 1 for k in range(K)]

_CACHE = {}


def _build():
    nc = bacc.Bacc('TRN2', target_bir_lowering=False, debug=False, num_devices=G)

    # ---- inputs (per-core data differs, program identical) ----
    inp8 = nc.dram_tensor("inp8", [128, 4, NPAD], F8, kind="ExternalInput")
    img_f = nc.dram_tensor("img_f", [2, 128, NFLAT], BF, kind="ExternalInput")
    img_s = nc.dram_tensor("img_s", [2, 128, NFLAT], BF, kind="ExternalInput")
    w1t = nc.dram_tensor("w1t", [128, 18, 2, 128], F8, kind="ExternalInput")
    w2t = nc.dram_tensor("w2t", [128, 9, 64], F8, kind="ExternalInput")
    w3t = nc.dram_tensor("w3t", [64, 9, 32], F8, kind="ExternalInput")
    w4t = nc.dram_tensor("w4t", [32, 9, 32], F8, kind="ExternalInput")
    b1d = nc.dram_tensor("b1d", [128, 1], F32, kind="ExternalInput")
    b2d = nc.dram_tensor("b2d", [64, 1], F32, kind="ExternalInput")
    b3d = nc.dram_tensor("b3d", [32, 1], F32, kind="ExternalInput")
    b4d = nc.dram_tensor("b4d", [18, 1], F32, kind="ExternalInput")
    i96d = nc.dram_tensor("i96d", [96, 256], F32, kind="ExternalInput")
    j96d = nc.dram_tensor("j96d", [96, 256], F32, kind="ExternalInput")
    seld = nc.dram_tensor("seld", [96, 16], BF, kind="ExternalInput")

    out_part = nc.dram_tensor("out_part", [2, 128, HW], BF, kind="ExternalOutput")

    with tile.TileContext(nc) as tc:
        with tc.tile_pool(name="consts", bufs=1) as consts, \
             tc.tile_pool(name="wpool", bufs=1) as wpool, \
             tc.tile_pool(name="hbufs", bufs=1) as hbufs, \
             tc.tile_pool(name="psum", bufs=2, space="PSUM") as psum_pool, \
             tc.tile_pool(name="dram", bufs=1, space="DRAM") as dram:

            # warp images (flat + shifted, mask pre-applied, 2 channel tiles)
            _early = contextlib.ExitStack()
            imgs = _early.enter_context(tc.tile_pool(name="imgs2", bufs=1))
            imf = [imgs.tile([128, NFLAT], BF, name=f"imf{c}") for c in range(2)]
            ims = [imgs.tile([128, NFLAT], BF, name=f"ims{c}") for c in range(2)]

            # ---- load weights / constants (w1 + inputs first on sync q) ----
            _c1 = contextlib.ExitStack()
            convin = _c1.enter_context(tc.tile_pool(name="convin", bufs=1))
            w1_t = convin.tile([128, 18, 2, 128], F8)
            nc.scalar.dma_start(w1_t[:], w1t[:, :, :, :])
            x8 = convin.tile([128, 4, NPAD], F8)
            for lo, hi in ((0, 1122), (1122, 2244), (2244, 3366), (3366, NPAD)):
                nc.sync.dma_start(x8[:, :, lo:hi], inp8[:, :, lo:hi])

            w2_t = wpool.tile([128, 9, 64], F8)
            w3_t = wpool.tile([64, 9, 32], F8)
            w4_t = wpool.tile([32, 9, 32], F8)
            b1_t = consts.tile([128, 1], F32)
            b2_t = consts.tile([64, 1], F32)
            b3_t = consts.tile([32, 1], F32)
            b4_t = consts.tile([18, 1], F32)
            i96 = consts.tile([96, 256], F32)
            j96 = consts.tile([96, 256], F32)
            sel = consts.tile([96, 16], BF)
            nc.scalar.dma_start(w2_t[:], w2t[:, :, :])
            nc.scalar.dma_start(w3_t[:], w3t[:, :, :])
            nc.scalar.dma_start(w4_t[:], w4t[:, :, :])
            nc.scalar.dma_start(b1_t[:], b1d[:, :])
            nc.scalar.dma_start(b2_t[:], b2d[:, :])
            nc.scalar.dma_start(b3_t[:], b3d[:, :])
            nc.scalar.dma_start(b4_t[:], b4d[:, :])
            nc.scalar.dma_start(i96[:], i96d[:, :])
            nc.scalar.dma_start(j96[:], j96d[:, :])
            nc.scalar.dma_start(sel[:], seld[:, :])

            # warmup collective: absorbs CC stream setup cost early
            ccw_in = dram.tile([1, 8], F32)
            ccw_out = dram.tile([1, 8], F32, addr_space="Shared")
            wseed = consts.tile([1, 8], F32)
            nc.vector.memset(wseed[:], 0.0)
            nc.gpsimd.dma_start(ccw_in[:], wseed[:])
            nc.gpsimd.collective_compute(
                "AllReduce", AluOp.add,
                replica_groups=[list(range(G))],
                ins=[ccw_in.opt()], outs=[ccw_out.opt()])

            # hidden activations: fp8, padded layout, dup copy shifted by 1
            h1 = hbufs.tile([128, 3, NPAD], F8)
            h2 = hbufs.tile([64, 3, NPAD], F8)
            h3 = hbufs.tile([32, 3, NPAD], F8)
            nc.vector.memset(h1[:], 0.0)
            nc.vector.memset(h2[:], 0.0)
            nc.vector.memset(h3[:], 0.0)

            ccs = hbufs.tile([6, HW], F32)     # exp(logits)
            oa18 = hbufs.tile([18, HW], F32)   # conv4 out: logits(6) + offsets(12)

            def conv234(src, wt, cout, drain):
                """fp8 conv via DoubleRow tap pairs (0,1),(3,4),(6,7) +
                plain-fp8 singles (2,5,8). src: [P, 2, NPAD] dup tile."""
                sv = src[:].rearrange("p t (r c) -> p t r c", c=PADW)
                for r in range(8):
                    ps = psum_pool.tile([cout, 512], F32, tag="convps", name="cp")
                    first = True
                    for t in (0, 3, 6):
                        di = t // 3
                        rhs = sv[:, 0:2, r * 8 + di:r * 8 + di + 8, 0:64]
                        nc.tensor.matmul(ps[:], wt[:, t:t + 2, :], rhs,
                                         start=first, stop=False,
                                         perf_mode=PM.DoubleRow,
                                         skip_group_check=True)
                        first = False
                    # taps (2,5) via copies 0 and 2 (the <<66 dup)
                    rhs = sv[:, 0:3:2, r * 8 + 0:r * 8 + 8, 2:66]
                    nc.tensor.matmul(ps[:], wt[:, 2:6:3, :], rhs,
                                     start=False, stop=False,
                                     perf_mode=PM.DoubleRow,
                                     skip_group_check=True)
                    rhs = sv[:, 0, r * 8 + 2:r * 8 + 10, 2:66]
                    nc.tensor.matmul(ps[:], wt[:, 8, :], rhs,
                                     start=False, stop=True,
                                     skip_group_check=True)
                    drain(r, ps)

            def drain_lrelu(dst, bias):
                dv0 = dst[:, 0, :].rearrange("p (r c) -> p r c", c=PADW)
                dv1 = dst[:, 1, :].rearrange("p (r c) -> p r c", c=PADW)
                dv2 = dst[:, 2, :].rearrange("p (r c) -> p r c", c=PADW)

                def d(r, ps):
                    nc.scalar.activation(dv0[:, r * 8 + 1:r * 8 + 9, 1:65],
                                         ps[:], ActFn.Lrelu,
                                         bias=bias[:, 0:1], scale=1.0 / WS,
                                         alpha=0.1)
                    nc.scalar.activation(dv1[:, r * 8 + 1:r * 8 + 9, 0:64],
                                         ps[:], ActFn.Lrelu,
                                         bias=bias[:, 0:1], scale=1.0 / WS,
                                         alpha=0.1)
                    nc.scalar.activation(dv2[:, r * 8:r * 8 + 8, 1:65],
                                         ps[:], ActFn.Lrelu,
                                         bias=bias[:, 0:1], scale=1.0 / WS,
                                         alpha=0.1)
                return d

            # ---- conv1: DoubleRow over input-channel slice pairs ----
            xv = x8[:].rearrange("p s (r c) -> p s r c", c=PADW)
            d1 = drain_lrelu(h1, b1_t)
            for r in range(8):
                ps = psum_pool.tile([128, 512], F32, tag="convps", name="cp1")
                i_mm = 0
                for t in range(9):
                    di, dj = t // 3, t % 3
                    for j in range(2):
                        rhs = xv[:, 2 * j:2 * j + 2,
                                 r * 8 + di:r * 8 + di + 8, dj:dj + 64]
                        nc.tensor.matmul(ps[:], w1_t[:, t * 2 + j, :, :], rhs,
                                         start=(i_mm == 0), stop=(i_mm == 17),
                                         perf_mode=PM.DoubleRow,
                                         skip_group_check=True)
                        i_mm += 1
                d1(r, ps)
            _c1.close()
            for c in range(2):
                nc.scalar.dma_start(imf[c][:], img_f[c, :, :])
                nc.scalar.dma_start(ims[c][:], img_s[c, :, :])

            conv234(h1, w2_t[:], 64, drain_lrelu(h2, b2_t))
            conv234(h2, w3_t[:], 32, drain_lrelu(h3, b3_t))

            # ---- conv4: drain, then exp() of the logit rows ----
            def d4(r, ps):
                sl = slice(r * 512, (r + 1) * 512)
                nc.scalar.activation(oa18[:, sl], ps[0:18, :], ActFn.Identity,
                                     bias=b4_t[:, 0:1], scale=1.0 / WS)
                nc.scalar.activation(ccs[:, sl], oa18[0:6, sl], ActFn.Exp)
            conv234(h3, w4_t[:], 32, d4)

            _late = contextlib.ExitStack()
            maps = _late.enter_context(tc.tile_pool(name="maps", bufs=1))
            mtmp = _late.enter_context(tc.tile_pool(name="mtmp", bufs=6))
            warp = _late.enter_context(tc.tile_pool(name="warp", bufs=3))

            # ---- softmax across groups (AllReduce of exp(logits)) ----
            cc_in = dram.tile([6, HW], F32)
            cc_out = dram.tile([6, HW], F32, addr_space="Shared")
            nc.scalar.dma_start(cc_in[:, 0:2048], ccs[:, 0:2048])
            nc.scalar.dma_start(cc_in[:, 2048:HW], ccs[:, 2048:HW])
            nc.gpsimd.collective_compute(
                "AllReduce", AluOp.add,
                replica_groups=[list(range(G))],
                ins=[cc_in.opt()], outs=[cc_out.opt()])

            # offsets to DRAM for the [96,256] reshape (parallel with CC)
            oa_d = dram.tile([12, HW], F32)
            nc.sync.dma_start(oa_d[:, 0:2048], oa18[6:18, 0:2048])
            nc.sync.dma_start(oa_d[:, 2048:HW], oa18[6:18, 2048:HW])

            # ---- [96, 256] map computation ----
            ox = maps.tile([96, 256], F32)
            oy = maps.tile([96, 256], F32)
            ex96 = maps.tile([96, 256], F32)
            tot96 = maps.tile([96, 256], F32)
            oav = oa_d[:].rearrange("a (q f) -> (a q) f", f=256)
            nc.sync.dma_start(ox[:], oav[0:96, :])
            nc.sync.dma_start(oy[:], oav[96:192, :])
            ccv_in = cc_in[:].rearrange("a (q f) -> (a q) f", f=256)
            ccv_out = cc_out[:].rearrange("a (q f) -> (a q) f", f=256)
            nc.scalar.dma_start(ex96[:], ccv_in[0:96, :])
            nc.sync.dma_start(tot96[:], ccv_out[0:96, :])

            def axis_maps(off_t, coord):
                """returns w[dv] weight tiles for dv in (-1, 0, 1)."""
                t1 = mtmp.tile([96, 256], F32, tag="t", name="t4")
                nc.vector.tensor_tensor(t1[:], off_t[:], coord[:], AluOp.add)
                x = mtmp.tile([96, 256], F32, tag="t", name="t5")
                nc.vector.tensor_scalar(x[:], t1[:], 64.0 / 63.0, -0.5,
                                        AluOp.mult, AluOp.add)
                xc = mtmp.tile([96, 256], F32, tag="t", name="t6")
                nc.vector.tensor_scalar(xc[:], x[:], 0.0, 63.0,
                                        AluOp.max, AluOp.min)
                ri = mtmp.tile([96, 256], I32, tag="ti", name="t7")
                nc.vector.tensor_copy(ri[:], xc[:])
                rf = mtmp.tile([96, 256], F32, tag="t", name="t8")
                nc.vector.tensor_copy(rf[:], ri[:])
                gt = mtmp.tile([96, 256], F32, tag="t", name="t9")
                nc.vector.tensor_tensor(gt[:], rf[:], xc[:], AluOp.is_gt)
                x0 = mtmp.tile([96, 256], F32, tag="t", name="t10")
                nc.vector.tensor_tensor(x0[:], rf[:], gt[:], AluOp.subtract)
                fx = mtmp.tile([96, 256], F32, tag="t", name="t11")
                nc.vector.tensor_tensor(fx[:], xc[:], x0[:], AluOp.subtract)
                x1 = mtmp.tile([96, 256], F32, tag="t", name="t12")
                nc.vector.tensor_scalar(x1[:], x0[:], 1.0, 63.0,
                                        AluOp.add, AluOp.min)
                d0 = mtmp.tile([96, 256], F32, tag="t", name="t13")
                nc.vector.tensor_tensor(d0[:], x0[:], coord[:], AluOp.subtract)
                d1_ = mtmp.tile([96, 256], F32, tag="t", name="t14")
                nc.vector.tensor_tensor(d1_[:], x1[:], coord[:], AluOp.subtract)
                fm = mtmp.tile([96, 256], F32, tag="t", name="t15")
                nc.vector.tensor_scalar(fm[:], fx[:], -1.0, 1.0,
                                        AluOp.mult, AluOp.add)
                ws = {}
                for dv in (-1.0, 0.0, 1.0):
                    a0 = mtmp.tile([96, 256], F32, tag="t", name="t16")
                    nc.vector.scalar_tensor_tensor(a0[:], d0[:], dv, fm[:],
                                                   AluOp.is_equal, AluOp.mult)
                    a1 = mtmp.tile([96, 256], F32, tag="t", name="t17")
                    nc.vector.scalar_tensor_tensor(a1[:], d1_[:], dv, fx[:],
                                                   AluOp.is_equal, AluOp.mult)
                    wv = maps.tile([96, 256], F32, name=f"w_{coord.name}_{int(dv)}")
                    nc.vector.tensor_tensor(wv[:], a0[:], a1[:], AluOp.add)
                    ws[int(dv)] = wv
                return ws

            wxs = axis_maps(ox, j96)
            wys = axis_maps(oy, i96)

            # attn = exp / allreduce-total (after CC completes)
            at = maps.tile([96, 256], F32)
            rc = mtmp.tile([96, 256], F32, tag="t", name="t3")
            nc.vector.reciprocal(rc[:], tot96[:])
            nc.vector.tensor_tensor(at[:], ex96[:], rc[:], AluOp.mult)

            prod = maps.tile([96, 2304], BF)
            for yi, dyv in enumerate((-1, 0, 1)):
                ad = mtmp.tile([96, 256], F32, tag="t", name="t18")
                nc.vector.tensor_tensor(ad[:], at[:], wys[dyv][:], AluOp.mult)
                for xi, dxv in enumerate((-1, 0, 1)):
                    di = yi * 3 + xi
                    nc.vector.tensor_tensor(prod[:, di * 256:(di + 1) * 256],
                                            ad[:], wxs[dxv][:], AluOp.mult)

            # K-sum via selection matmul -> Wd [16, 2304]
            wps = psum_pool.tile([16, 2304], F32, tag="wdps", bufs=1, name="wdps")
            wd16 = maps.tile([16, 2304], BF)
            wd_d = dram.tile([16, 2304], BF)
            for c0 in range(0, 2304, 512):
                cn = min(512, 2304 - c0)
                nc.tensor.matmul(wps[:, c0:c0 + cn], sel[:], prod[:, c0:c0 + cn],
                                 start=True, stop=True)
                nc.scalar.activation(wd16[:, c0:c0 + cn], wps[:, c0:c0 + cn],
                                     ActFn.Copy)
                nc.gpsimd.dma_start(wd_d[:, c0:c0 + cn], wd16[:, c0:c0 + cn])

            # ---- warp: out[c,p] = sum_d Wd[p] * img[c, p+d] ----
            bq = [nc.sync, nc.scalar, nc.gpsimd]
            acc = [None, None]
            for di9 in range(9):
                dy, dx = di9 // 3 - 1, di9 % 3 - 1
                wdb = warp.tile([128, HW], BF, tag="wdb", bufs=2, name="t20")
                src = wd_d[0:16, di9 * 256:(di9 + 1) * 256]
                bq[di9 % 3].dma_start(wdb[:], src.partition_broadcast(128))
                for c in range(2):
                    base = GUARD + 64 * dy
                    if dx == 0:
                        img_ap = imf[c][:, base:base + HW]
                    elif dx == 1:
                        img_ap = ims[c][:, base:base + HW]
                    else:
                        img_ap = ims[c][:, base - 2:base - 2 + HW]
                    if acc[c] is None:
                        acc[c] = warp.tile([128, HW], BF, tag=f"acc{c}", bufs=2, name="t21")
                        nc.vector.tensor_tensor(acc[c][:], img_ap, wdb[:], AluOp.mult)
                    else:
                        pr = warp.tile([128, HW], BF, tag="pr", bufs=2, name="t22")
                        nc.vector.tensor_tensor(pr[:], img_ap, wdb[:], AluOp.mult)
                        nacc = warp.tile([128, HW], BF, tag=f"acc{c}", bufs=2, name="t23")
                        nc.vector.tensor_tensor(nacc[:], acc[c][:], pr[:], AluOp.add)
                        acc[c] = nacc

            for c in range(2):
                nc.sync.dma_start(out_part[c, :, :], acc[c][:])
            _late.close()
            _early.close()

    nc.compile()
    return nc


def _prep_inputs(gar_feat, cond_feat, mask, W1, b1, W2, b2, W3, b3, W4, b4):
    """Host-side prep: returns list of 8 per-core input dicts."""
    gar = np.asarray(gar_feat, np.float32)[0]      # [256, 64, 64]
    cond = np.asarray(cond_feat, np.float32)[0]
    maskf = np.asarray(mask, np.float32)[0]        # [G, 256]

    inp = np.concatenate([gar, cond], axis=0)      # [512, 64, 64]
    inp_pad = np.zeros((C_IN, PADW, PADW), np.float32)
    inp_pad[:, 1:-1, 1:-1] = inp
    # [128, 4, NPAD]: partition p, slice s -> channel s*128+p
    inp8 = inp_pad.reshape(4, 128, NPAD).transpose(1, 0, 2).astype(FP8)

    i_idx = (np.arange(HW, dtype=np.float32) // W).reshape(16, 256)
    j_idx = (np.arange(HW, dtype=np.float32) % W).reshape(16, 256)
    i96 = np.tile(i_idx, (6, 1)).astype(np.float32)
    j96 = np.tile(j_idx, (6, 1)).astype(np.float32)
    sel = np.zeros((96, 16), np.float32)
    sel[np.arange(96), np.arange(96) % 16] = 1.0
    sel = sel.astype(BF16)

    per_core = []
    for g in range(G):
        w1g = np.asarray(W1[g], np.float32) * WS   # [128, 512, 3, 3]
        w2g = np.asarray(W2[g], np.float32) * WS   # [64, 128, 3, 3]
        w3g = np.asarray(W3[g], np.float32) * WS   # [32, 64, 3, 3]
        w4g = (np.asarray(W4[g], np.float32) * WS)[PERM4]  # [18, 32, 3, 3]
        b4g = np.asarray(b4[g], np.float32)[PERM4]

        # w1T[p, t*2+j, s, o] = w1g[o, (2j+s)*128+p, di, dj]
        w1T = np.zeros((128, 18, 2, 128), np.float32)
        for t in range(9):
            di, dj = t // 3, t % 3
            for jj in range(2):
                for s in range(2):
                    sl = 2 * jj + s
                    w1T[:, t * 2 + jj, s, :] = w1g[:, sl * 128:(sl + 1) * 128, di, dj].T
        w2T = np.zeros((128, 9, 64), np.float32)
        w3T = np.zeros((64, 9, 32), np.float32)
        w4T = np.zeros((32, 9, 32), np.float32)
        for t in range(9):
            di, dj = t // 3, t % 3
            w2T[:, t, :] = w2g[:, :, di, dj].T
            w3T[:, t, :] = w3g[:, :, di, dj].T
            w4T[:, t, 0:18] = w4g[:, :, di, dj].T

        garm = gar * maskf[g][:, None, None]   # fold group mask into warp img
        img_flat = np.zeros((2, 128, NFLAT), np.float32)
        img_flat[:, :, GUARD:GUARD + HW] = garm.reshape(2, 128, HW)
        img_shift = np.zeros((2, 128, NFLAT), np.float32)
        img_shift[:, :, :-1] = img_flat[:, :, 1:]

        per_core.append({
            "inp8": inp8,
            "img_f": img_flat.astype(BF16),
            "img_s": img_shift.astype(BF16),
            "w1t": w1T.astype(FP8),
            "w2t": w2T.astype(FP8),
            "w3t": w3T.astype(FP8),
            "w4t": w4T.astype(FP8),
            "b1d": np.asarray(b1[g], np.float32).reshape(128, 1),
            "b2d": np.asarray(b2[g], np.float32).reshape(64, 1),
            "b3d": np.asarray(b3[g], np.float32).reshape(32, 1),
            "b4d": b4g.reshape(18, 1),
            "i96d": i96, "j96d": j96, "seld": sel,
        })
    return per_core


def _get_nc():
    if "nc" not in _CACHE:
        _CACHE["nc"] = _build()
    return _CACHE["nc"]


def run_cores(inputs, trace=False):
    nc = _get_nc()
    in_maps = _prep_inputs(**inputs)
    res = run_bass_kernel_spmd(nc, in_maps, core_ids=list(range(G)), trace=trace)
    return res


def kernel(**inputs) -> np.ndarray:
    res = run_cores(inputs, trace=False)
    out = np.zeros((C_FEAT, HW), np.float32)
    for r in res.results:
        out += r["out_part"].reshape(C_FEAT, HW).astype(np.float32)
    return out.reshape(1, C_FEAT, H, W)


# revision 19
# speedup vs baseline: 1.0727x; 1.0727x over previous
"""Trainium2 Bass kernel for nn_DSDModules_57681410785615 (sparse_attention).

Strategy (expert-parallel over the group axis G=8, one group per NeuronCore):
  - Each core runs its group's 4-layer 3x3 conv stack as shifted matmuls in
    fp8e4 with DoubleRow perf mode (2 contraction rows per partition):
    conv1 pairs input-channel slices; conv2..4 pair adjacent taps using a
    second, 1-pixel-shifted fp8 copy of the activations (written by the
    Scalar engine, which also fuses bias + leaky-relu + 1/64 weight-descale).
  - conv4 drain fuses exp() for the logit rows; cross-group softmax via an
    8-core AllReduce of exp(logits), overlapped with the offset->bilinear
    weight map computation on DVE.
  - The bilinear warp is a 9-point spatially-varying stencil: per-pixel
    stencil weights fold the K=6 taps, bilinear fractions and softmax attn
    into 9 maps, reduced over K on the TensorEngine and broadcast across
    partitions via a DRAM round-trip DMA (spread over queues).
  - The group mask is pre-multiplied into the warp source image on the host.
  - Each core DMAs out its masked partial [256, 64*64] bf16; the host sums
    the 8 partials (the reference's sum over groups) in fp32.

Self-contained: hardcodes all shapes; no file reads.
"""
import sys
import contextlib

for _p in ('/opt/trn_rl_repo', '/opt/trn_rl_repo/concourse'):
    if _p not in sys.path:
        sys.path.insert(0, _p)

import numpy as np
import ml_dtypes

import concourse.bass as bass
import concourse.mybir as mybir
import concourse.tile as tile
from concourse import bacc
from concourse.bass_utils import run_bass_kernel_spmd

BF16 = ml_dtypes.bfloat16
FP8 = ml_dtypes.float8_e4m3
F32 = mybir.dt.float32
BF = mybir.dt.bfloat16
F8 = mybir.dt.float8e4
I32 = mybir.dt.int32
PM = mybir.MatmulPerfMode

G, K, C_IN, C_FEAT, H, W, B = 8, 6, 512, 256, 64, 64, 1
HW = H * W                  # 4096
PADW = 66                   # padded conv row width
NPAD = PADW * PADW          # 4356 padded conv pixels
GUARD = 66                  # flat warp-image guard elements (even)
NFLAT = GUARD + HW + GUARD  # 4228
WS = 64.0                   # fp8 weight scale
AluOp = mybir.AluOpType
ActFn = mybir.ActivationFunctionType

# conv4 output channel permutation: [logit_k (6), offx_k (6), offy_k (6)]
PERM4 = [12 + k for k in range(K)] + [2 * k for k in range(K)] + [2 * k + 1 for k in range(K)]

_CACHE = {}


def _build():
    nc = bacc.Bacc('TRN2', target_bir_lowering=False, debug=False, num_devices=G)

    # ---- inputs (per-core data differs, program identical) ----
    inp8 = nc.dram_tensor("inp8", [128, 4, NPAD], F8, kind="ExternalInput")
    img_f = nc.dram_tensor("img_f", [2, 128, NFLAT], BF, kind="ExternalInput")
    img_s = nc.dram_tensor("img_s", [2, 128, NFLAT], BF, kind="ExternalInput")
    w1t = nc.dram_tensor("w1t", [128, 18, 2, 128], F8, kind="ExternalInput")
    w2t = nc.dram_tensor("w2t", [128, 9, 64], F8, kind="ExternalInput")
    w3t = nc.dram_tensor("w3t", [64, 9, 32], F8, kind="ExternalInput")
    w4t = nc.dram_tensor("w4t", [32, 9, 32], F8, kind="ExternalInput")
    b1d = nc.dram_tensor("b1d", [128, 1], F32, kind="ExternalInput")
    b2d = nc.dram_tensor("b2d", [64, 1], F32, kind="ExternalInput")
    b3d = nc.dram_tensor("b3d", [32, 1], F32, kind="ExternalInput")
    b4d = nc.dram_tensor("b4d", [18, 1], F32, kind="ExternalInput")
    i96d = nc.dram_tensor("i96d", [96, 256], F32, kind="ExternalInput")
    j96d = nc.dram_tensor("j96d", [96, 256], F32, kind="ExternalInput")
    seld = nc.dram_tensor("seld", [96, 16], BF, kind="ExternalInput")

    out_part = nc.dram_tensor("out_part", [2, 128, HW], BF, kind="ExternalOutput")

    with tile.TileContext(nc) as tc:
        with tc.tile_pool(name="consts", bufs=1) as consts, \
             tc.tile_pool(name="wpool", bufs=1) as wpool, \
             tc.tile_pool(name="hbufs", bufs=1) as hbufs, \
             tc.tile_pool(name="psum", bufs=3, space="PSUM") as psum_pool, \
             tc.tile_pool(name="dram", bufs=1, space="DRAM") as dram:

            # warp images (flat + shifted, mask pre-applied, 2 channel tiles)
            _early = contextlib.ExitStack()
            imgs = _early.enter_context(tc.tile_pool(name="imgs2", bufs=1))
            imf = [imgs.tile([128, NFLAT], BF, name=f"imf{c}") for c in range(2)]
            ims = [imgs.tile([128, NFLAT], BF, name=f"ims{c}") for c in range(2)]

            # ---- load weights / constants (w1 + inputs first on sync q) ----
            _c1 = contextlib.ExitStack()
            convin = _c1.enter_context(tc.tile_pool(name="convin", bufs=1))
            w1_t = convin.tile([128, 18, 2, 128], F8)
            nc.scalar.dma_start(w1_t[:], w1t[:, :, :, :])
            x8 = convin.tile([128, 4, NPAD], F8)
            for lo, hi in ((0, 1122), (1122, 2244), (2244, 3366), (3366, NPAD)):
                nc.sync.dma_start(x8[:, :, lo:hi], inp8[:, :, lo:hi])

            w2_t = wpool.tile([128, 9, 64], F8)
            w3_t = wpool.tile([64, 9, 32], F8)
            w4_t = wpool.tile([32, 9, 32], F8)
            b1_t = consts.tile([128, 1], F32)
            b2_t = consts.tile([64, 1], F32)
            b3_t = consts.tile([32, 1], F32)
            b4_t = consts.tile([18, 1], F32)
            i96 = consts.tile([96, 256], F32)
            j96 = consts.tile([96, 256], F32)
            sel = consts.tile([96, 16], BF)
            nc.scalar.dma_start(w2_t[:], w2t[:, :, :])
            nc.scalar.dma_start(w3_t[:], w3t[:, :, :])
            nc.scalar.dma_start(w4_t[:], w4t[:, :, :])
            nc.scalar.dma_start(b1_t[:], b1d[:, :])
            nc.scalar.dma_start(b2_t[:], b2d[:, :])
            nc.scalar.dma_start(b3_t[:], b3d[:, :])
            nc.scalar.dma_start(b4_t[:], b4d[:, :])
            nc.scalar.dma_start(i96[:], i96d[:, :])
            nc.scalar.dma_start(j96[:], j96d[:, :])
            nc.scalar.dma_start(sel[:], seld[:, :])

            # warmup collective: absorbs CC stream setup cost early
            ccw_in = dram.tile([1, 8], F32)
            ccw_out = dram.tile([1, 8], F32, addr_space="Shared")
            wseed = consts.tile([1, 8], F32)
            nc.vector.memset(wseed[:], 0.0)
            nc.gpsimd.dma_start(ccw_in[:], wseed[:])
            nc.gpsimd.collective_compute(
                "AllReduce", AluOp.add,
                replica_groups=[list(range(G))],
                ins=[ccw_in.opt()], outs=[ccw_out.opt()])

            # hidden activations: fp8, padded layout, dup copy shifted by 1
            h1 = hbufs.tile([128, 3, NPAD], F8)
            h2 = hbufs.tile([64, 3, NPAD], F8)
            h3 = hbufs.tile([32, 3, NPAD], F8)
            nc.vector.memset(h1[:], 0.0)
            nc.vector.memset(h2[:], 0.0)
            nc.vector.memset(h3[:], 0.0)

            ccs = hbufs.tile([6, HW], F32)     # exp(logits)
            oa18 = hbufs.tile([18, HW], F32)   # conv4 out: logits(6) + offsets(12)

            def conv234(src, wt, cout, drain):
                """fp8 conv via DoubleRow tap pairs (0,1),(3,4),(6,7) +
                plain-fp8 singles (2,5,8). src: [P, 2, NPAD] dup tile."""
                sv = src[:].rearrange("p t (r c) -> p t r c", c=PADW)
                for r in range(8):
                    ps = psum_pool.tile([cout, 512], F32, tag="convps", name="cp")
                    first = True
                    for t in (0, 3, 6):
                        di = t // 3
                        rhs = sv[:, 0:2, r * 8 + di:r * 8 + di + 8, 0:64]
                        nc.tensor.matmul(ps[:], wt[:, t:t + 2, :], rhs,
                                         start=first, stop=False,
                                         perf_mode=PM.DoubleRow,
                                         skip_group_check=True)
                        first = False
                    # taps (2,5) via copies 0 and 2 (the <<66 dup)
                    rhs = sv[:, 0:3:2, r * 8 + 0:r * 8 + 8, 2:66]
                    nc.tensor.matmul(ps[:], wt[:, 2:6:3, :], rhs,
                                     start=False, stop=False,
                                     perf_mode=PM.DoubleRow,
                                     skip_group_check=True)
                    rhs = sv[:, 0, r * 8 + 2:r * 8 + 10, 2:66]
                    nc.tensor.matmul(ps[:], wt[:, 8, :], rhs,
                                     start=False, stop=True,
                                     skip_group_check=True)
                    drain(r, ps)

            def drain_lrelu(dst, bias):
                dv0 = dst[:, 0, :].rearrange("p (r c) -> p r c", c=PADW)
                dv1 = dst[:, 1, :].rearrange("p (r c) -> p r c", c=PADW)
                dv2 = dst[:, 2, :].rearrange("p (r c) -> p r c", c=PADW)

                def d(r, ps):
                    nc.scalar.activation(dv0[:, r * 8 + 1:r * 8 + 9, 1:65],
                                         ps[:], ActFn.Lrelu,
                                         bias=bias[:, 0:1], scale=1.0 / WS,
                                         alpha=0.1)
                    nc.scalar.activation(dv1[:, r * 8 + 1:r * 8 + 9, 0:64],
                                         ps[:], ActFn.Lrelu,
                                         bias=bias[:, 0:1], scale=1.0 / WS,
                                         alpha=0.1)
                    nc.scalar.activation(dv2[:, r * 8:r * 8 + 8, 1:65],
                                         ps[:], ActFn.Lrelu,
                                         bias=bias[:, 0:1], scale=1.0 / WS,
                                         alpha=0.1)
                return d

            # ---- conv1: DoubleRow over input-channel slice pairs ----
            xv = x8[:].rearrange("p s (r c) -> p s r c", c=PADW)
            d1 = drain_lrelu(h1, b1_t)
            for r in range(8):
                ps = psum_pool.tile([128, 512], F32, tag="convps", name="cp1")
                i_mm = 0
                for t in range(9):
                    di, dj = t // 3, t % 3
                    for j in range(2):
                        rhs = xv[:, 2 * j:2 * j + 2,
                                 r * 8 + di:r * 8 + di + 8, dj:dj + 64]
                        nc.tensor.matmul(ps[:], w1_t[:, t * 2 + j, :, :], rhs,
                                         start=(i_mm == 0), stop=(i_mm == 17),
                                         perf_mode=PM.DoubleRow,
                                         skip_group_check=True)
                        i_mm += 1
                d1(r, ps)
            _c1.close()
            for c in range(2):
                nc.scalar.dma_start(imf[c][:], img_f[c, :, :])
                nc.scalar.dma_start(ims[c][:], img_s[c, :, :])

            conv234(h1, w2_t[:], 64, drain_lrelu(h2, b2_t))
            conv234(h2, w3_t[:], 32, drain_lrelu(h3, b3_t))

            # ---- conv4: drain, then exp() of the logit rows ----
            def d4(r, ps):
                sl = slice(r * 512, (r + 1) * 512)
                nc.scalar.activation(oa18[:, sl], ps[0:18, :], ActFn.Identity,
                                     bias=b4_t[:, 0:1], scale=1.0 / WS)
                nc.scalar.activation(ccs[:, sl], oa18[0:6, sl], ActFn.Exp)
            conv234(h3, w4_t[:], 32, d4)

            _late = contextlib.ExitStack()
            maps = _late.enter_context(tc.tile_pool(name="maps", bufs=1))
            mtmp = _late.enter_context(tc.tile_pool(name="mtmp", bufs=6))
            warp = _late.enter_context(tc.tile_pool(name="warp", bufs=3))

            # ---- softmax across groups (AllReduce of exp(logits)) ----
            cc_in = dram.tile([6, HW], F32)
            cc_out = dram.tile([6, HW], F32, addr_space="Shared")
            nc.scalar.dma_start(cc_in[:, 0:2048], ccs[:, 0:2048])
            nc.scalar.dma_start(cc_in[:, 2048:HW], ccs[:, 2048:HW])
            nc.gpsimd.collective_compute(
                "AllReduce", AluOp.add,
                replica_groups=[list(range(G))],
                ins=[cc_in.opt()], outs=[cc_out.opt()])

            # offsets to DRAM for the [96,256] reshape (parallel with CC)
            oa_d = dram.tile([12, HW], F32)
            nc.sync.dma_start(oa_d[:, 0:2048], oa18[6:18, 0:2048])
            nc.sync.dma_start(oa_d[:, 2048:HW], oa18[6:18, 2048:HW])

            # ---- [96, 256] map computation ----
            ox = maps.tile([96, 256], F32)
            oy = maps.tile([96, 256], F32)
            ex96 = maps.tile([96, 256], F32)
            tot96 = maps.tile([96, 256], F32)
            oav = oa_d[:].rearrange("a (q f) -> (a q) f", f=256)
            nc.sync.dma_start(ox[:], oav[0:96, :])
            nc.sync.dma_start(oy[:], oav[96:192, :])
            ccv_in = cc_in[:].rearrange("a (q f) -> (a q) f", f=256)
            ccv_out = cc_out[:].rearrange("a (q f) -> (a q) f", f=256)
            nc.scalar.dma_start(ex96[:], ccv_in[0:96, :])
            nc.sync.dma_start(tot96[:], ccv_out[0:96, :])

            def axis_maps(off_t, coord):
                """returns w[dv] weight tiles for dv in (-1, 0, 1)."""
                t1 = mtmp.tile([96, 256], F32, tag="t", name="t4")
                nc.vector.tensor_tensor(t1[:], off_t[:], coord[:], AluOp.add)
                x = mtmp.tile([96, 256], F32, tag="t", name="t5")
                nc.vector.tensor_scalar(x[:], t1[:], 64.0 / 63.0, -0.5,
                                        AluOp.mult, AluOp.add)
                xc = mtmp.tile([96, 256], F32, tag="t", name="t6")
                nc.vector.tensor_scalar(xc[:], x[:], 0.0, 63.0,
                                        AluOp.max, AluOp.min)
                ri = mtmp.tile([96, 256], I32, tag="ti", name="t7")
                nc.vector.tensor_copy(ri[:], xc[:])
                rf = mtmp.tile([96, 256], F32, tag="t", name="t8")
                nc.vector.tensor_copy(rf[:], ri[:])
                gt = mtmp.tile([96, 256], F32, tag="t", name="t9")
                nc.vector.tensor_tensor(gt[:], rf[:], xc[:], AluOp.is_gt)
                x0 = mtmp.tile([96, 256], F32, tag="t", name="t10")
                nc.vector.tensor_tensor(x0[:], rf[:], gt[:], AluOp.subtract)
                fx = mtmp.tile([96, 256], F32, tag="t", name="t11")
                nc.vector.tensor_tensor(fx[:], xc[:], x0[:], AluOp.subtract)
                x1 = mtmp.tile([96, 256], F32, tag="t", name="t12")
                nc.vector.tensor_scalar(x1[:], x0[:], 1.0, 63.0,
                                        AluOp.add, AluOp.min)
                d0 = mtmp.tile([96, 256], F32, tag="t", name="t13")
                nc.vector.tensor_tensor(d0[:], x0[:], coord[:], AluOp.subtract)
                d1_ = mtmp.tile([96, 256], F32, tag="t", name="t14")
                nc.vector.tensor_tensor(d1_[:], x1[:], coord[:], AluOp.subtract)
                fm = mtmp.tile([96, 256], F32, tag="t", name="t15")
                nc.vector.tensor_scalar(fm[:], fx[:], -1.0, 1.0,
                                        AluOp.mult, AluOp.add)
                ws = {}
                for dv in (-1.0, 0.0, 1.0):
                    a0 = mtmp.tile([96, 256], F32, tag="t", name="t16")
                    nc.vector.scalar_tensor_tensor(a0[:], d0[:], dv, fm[:],
                                                   AluOp.is_equal, AluOp.mult)
                    a1 = mtmp.tile([96, 256], F32, tag="t", name="t17")
                    nc.vector.scalar_tensor_tensor(a1[:], d1_[:], dv, fx[:],
                                                   AluOp.is_equal, AluOp.mult)
                    wv = maps.tile([96, 256], F32, name=f"w_{coord.name}_{int(dv)}")
                    nc.vector.tensor_tensor(wv[:], a0[:], a1[:], AluOp.add)
                    ws[int(dv)] = wv
                return ws

            wxs = axis_maps(ox, j96)
            wys = axis_maps(oy, i96)

            # attn = exp / allreduce-total (after CC completes)
            at = maps.tile([96, 256], F32)
            rc = mtmp.tile([96, 256], F32, tag="t", name="t3")
            nc.vector.reciprocal(rc[:], tot96[:])
            nc.vector.tensor_tensor(at[:], ex96[:], rc[:], AluOp.mult)

            prod = maps.tile([96, 2304], BF)
            for yi, dyv in enumerate((-1, 0, 1)):
                ad = mtmp.tile([96, 256], F32, tag="t", name="t18")
                nc.vector.tensor_tensor(ad[:], at[:], wys[dyv][:], AluOp.mult)
                for xi, dxv in enumerate((-1, 0, 1)):
                    di = yi * 3 + xi
                    nc.vector.tensor_tensor(prod[:, di * 256:(di + 1) * 256],
                                            ad[:], wxs[dxv][:], AluOp.mult)

            # K-sum via selection matmul -> Wd [16, 2304]
            wps = psum_pool.tile([16, 2304], F32, tag="wdps", bufs=1, name="wdps")
            wd16 = maps.tile([16, 2304], BF)
            wd_d = dram.tile([16, 2304], BF)
            for c0 in range(0, 2304, 512):
                cn = min(512, 2304 - c0)
                nc.tensor.matmul(wps[:, c0:c0 + cn], sel[:], prod[:, c0:c0 + cn],
                                 start=True, stop=True)
                nc.scalar.activation(wd16[:, c0:c0 + cn], wps[:, c0:c0 + cn],
                                     ActFn.Copy)
                nc.gpsimd.dma_start(wd_d[:, c0:c0 + cn], wd16[:, c0:c0 + cn])

            # ---- warp: out[c,p] = sum_d Wd[p] * img[c, p+d] ----
            bq = [nc.sync, nc.scalar, nc.gpsimd]
            acc = [None, None]
            for di9 in range(9):
                dy, dx = di9 // 3 - 1, di9 % 3 - 1
                wdb = warp.tile([128, HW], BF, tag="wdb", bufs=2, name="t20")
                src = wd_d[0:16, di9 * 256:(di9 + 1) * 256]
                bq[di9 % 3].dma_start(wdb[:], src.partition_broadcast(128))
                for c in range(2):
                    base = GUARD + 64 * dy
                    if dx == 0:
                        img_ap = imf[c][:, base:base + HW]
                    elif dx == 1:
                        img_ap = ims[c][:, base:base + HW]
                    else:
                        img_ap = ims[c][:, base - 2:base - 2 + HW]
                    if acc[c] is None:
                        acc[c] = warp.tile([128, HW], BF, tag=f"acc{c}", bufs=2, name="t21")
                        nc.vector.tensor_tensor(acc[c][:], img_ap, wdb[:], AluOp.mult)
                    else:
                        pr = warp.tile([128, HW], BF, tag="pr", bufs=2, name="t22")
                        nc.vector.tensor_tensor(pr[:], img_ap, wdb[:], AluOp.mult)
                        nacc = warp.tile([128, HW], BF, tag=f"acc{c}", bufs=2, name="t23")
                        nc.vector.tensor_tensor(nacc[:], acc[c][:], pr[:], AluOp.add)
                        acc[c] = nacc

            for c in range(2):
                nc.sync.dma_start(out_part[c, :, :], acc[c][:])
            _late.close()
            _early.close()

    nc.compile()
    return nc


def _prep_inputs(gar_feat, cond_feat, mask, W1, b1, W2, b2, W3, b3, W4, b4):
    """Host-side prep: returns list of 8 per-core input dicts."""
    gar = np.asarray(gar_feat, np.float32)[0]      # [256, 64, 64]
    cond = np.asarray(cond_feat, np.float32)[0]
    maskf = np.asarray(mask, np.float32)[0]        # [G, 256]

    inp = np.concatenate([gar, cond], axis=0)      # [512, 64, 64]
    inp_pad = np.zeros((C_IN, PADW, PADW), np.float32)
    inp_pad[:, 1:-1, 1:-1] = inp
    # [128, 4, NPAD]: partition p, slice s -> channel s*128+p
    inp8 = inp_pad.reshape(4, 128, NPAD).transpose(1, 0, 2).astype(FP8)

    i_idx = (np.arange(HW, dtype=np.float32) // W).reshape(16, 256)
    j_idx = (np.arange(HW, dtype=np.float32) % W).reshape(16, 256)
    i96 = np.tile(i_idx, (6, 1)).astype(np.float32)
    j96 = np.tile(j_idx, (6, 1)).astype(np.float32)
    sel = np.zeros((96, 16), np.float32)
    sel[np.arange(96), np.arange(96) % 16] = 1.0
    sel = sel.astype(BF16)

    per_core = []
    for g in range(G):
        w1g = np.asarray(W1[g], np.float32) * WS   # [128, 512, 3, 3]
        w2g = np.asarray(W2[g], np.float32) * WS   # [64, 128, 3, 3]
        w3g = np.asarray(W3[g], np.float32) * WS   # [32, 64, 3, 3]
        w4g = (np.asarray(W4[g], np.float32) * WS)[PERM4]  # [18, 32, 3, 3]
        b4g = np.asarray(b4[g], np.float32)[PERM4]

        # w1T[p, t*2+j, s, o] = w1g[o, (2j+s)*128+p, di, dj]
        w1T = np.zeros((128, 18, 2, 128), np.float32)
        for t in range(9):
            di, dj = t // 3, t % 3
            for jj in range(2):
                for s in range(2):
                    sl = 2 * jj + s
                    w1T[:, t * 2 + jj, s, :] = w1g[:, sl * 128:(sl + 1) * 128, di, dj].T
        w2T = np.zeros((128, 9, 64), np.float32)
        w3T = np.zeros((64, 9, 32), np.float32)
        w4T = np.zeros((32, 9, 32), np.float32)
        for t in range(9):
            di, dj = t // 3, t % 3
            w2T[:, t, :] = w2g[:, :, di, dj].T
            w3T[:, t, :] = w3g[:, :, di, dj].T
            w4T[:, t, 0:18] = w4g[:, :, di, dj].T

        garm = gar * maskf[g][:, None, None]   # fold group mask into warp img
        img_flat = np.zeros((2, 128, NFLAT), np.float32)
        img_flat[:, :, GUARD:GUARD + HW] = garm.reshape(2, 128, HW)
        img_shift = np.zeros((2, 128, NFLAT), np.float32)
        img_shift[:, :, :-1] = img_flat[:, :, 1:]

        per_core.append({
            "inp8": inp8,
            "img_f": img_flat.astype(BF16),
            "img_s": img_shift.astype(BF16),
            "w1t": w1T.astype(FP8),
            "w2t": w2T.astype(FP8),
            "w3t": w3T.astype(FP8),
            "w4t": w4T.astype(FP8),
            "b1d": np.asarray(b1[g], np.float32).reshape(128, 1),
            "b2d": np.asarray(b2[g], np.float32).reshape(64, 1),
            "b3d": np.asarray(b3[g], np.float32).reshape(32, 1),
            "b4d": b4g.reshape(18, 1),
            "i96d": i96, "j96d": j96, "seld": sel,
        })
    return per_core


def _get_nc():
    if "nc" not in _CACHE:
        _CACHE["nc"] = _build()
    return _CACHE["nc"]


def run_cores(inputs, trace=False):
    nc = _get_nc()
    in_maps = _prep_inputs(**inputs)
    res = run_bass_kernel_spmd(nc, in_maps, core_ids=list(range(G)), trace=trace)
    return res


def kernel(**inputs) -> np.ndarray:
    res = run_cores(inputs, trace=False)
    out = np.zeros((C_FEAT, HW), np.float32)
    for r in res.results:
        out += r["out_part"].reshape(C_FEAT, HW).astype(np.float32)
    return out.reshape(1, C_FEAT, H, W)


# revision 20
# speedup vs baseline: 1.2135x; 1.1313x over previous
"""Trainium2 Bass kernel for nn_DSDModules_57681410785615 (sparse_attention).

Strategy (expert-parallel over the group axis G=8, one group per NeuronCore):
  - Each core runs its group's 4-layer 3x3 conv stack as shifted matmuls in
    fp8e4 with DoubleRow perf mode (2 contraction rows per partition):
    conv1 pairs input-channel slices; conv2..4 pair adjacent taps using a
    second, 1-pixel-shifted fp8 copy of the activations (written by the
    Scalar engine, which also fuses bias + leaky-relu + 1/64 weight-descale).
  - conv4 drain fuses exp() for the logit rows; cross-group softmax via an
    8-core AllReduce of exp(logits), overlapped with the offset->bilinear
    weight map computation on DVE.
  - The bilinear warp is a 9-point spatially-varying stencil: per-pixel
    stencil weights fold the K=6 taps, bilinear fractions and softmax attn
    into 9 maps, reduced over K on the TensorEngine and broadcast across
    partitions via a DRAM round-trip DMA (spread over queues).
  - The group mask is pre-multiplied into the warp source image on the host.
  - Each core DMAs out its masked partial [256, 64*64] bf16; the host sums
    the 8 partials (the reference's sum over groups) in fp32.

Self-contained: hardcodes all shapes; no file reads.
"""
import sys
import contextlib

for _p in ('/opt/trn_rl_repo', '/opt/trn_rl_repo/concourse'):
    if _p not in sys.path:
        sys.path.insert(0, _p)

import numpy as np
import ml_dtypes

import concourse.bass as bass
import concourse.mybir as mybir
import concourse.tile as tile
from concourse import bacc
from concourse.bass_utils import run_bass_kernel_spmd

BF16 = ml_dtypes.bfloat16
FP8 = ml_dtypes.float8_e4m3
F32 = mybir.dt.float32
BF = mybir.dt.bfloat16
F8 = mybir.dt.float8e4
I32 = mybir.dt.int32
PM = mybir.MatmulPerfMode

G, K, C_IN, C_FEAT, H, W, B = 8, 6, 512, 256, 64, 64, 1
HW = H * W                  # 4096
PADW = 66                   # padded conv row width
NPAD = PADW * PADW          # 4356 padded conv pixels
GUARD = 66                  # flat warp-image guard elements (even)
NFLAT = GUARD + HW + GUARD  # 4228
WS = 64.0                   # fp8 weight scale
AluOp = mybir.AluOpType
ActFn = mybir.ActivationFunctionType

# conv4 output channel permutation: [logit_k (6), offx_k (6), offy_k (6)]
PERM4 = [12 + k for k in range(K)] + [2 * k for k in range(K)] + [2 * k + 1 for k in range(K)]

_CACHE = {}


def _build():
    nc = bacc.Bacc('TRN2', target_bir_lowering=False, debug=False, num_devices=G)

    # ---- inputs (per-core data differs, program identical) ----
    inp8 = nc.dram_tensor("inp8", [128, 4, NPAD], F8, kind="ExternalInput")
    img_f = nc.dram_tensor("img_f", [2, 128, NFLAT], BF, kind="ExternalInput")
    img_s = nc.dram_tensor("img_s", [2, 128, NFLAT], BF, kind="ExternalInput")
    w1t = nc.dram_tensor("w1t", [128, 18, 2, 128], F8, kind="ExternalInput")
    w2t = nc.dram_tensor("w2t", [128, 9, 64], F8, kind="ExternalInput")
    w3t = nc.dram_tensor("w3t", [64, 9, 32], F8, kind="ExternalInput")
    w4t = nc.dram_tensor("w4t", [32, 9, 32], F8, kind="ExternalInput")
    b1d = nc.dram_tensor("b1d", [128, 1], F32, kind="ExternalInput")
    b2d = nc.dram_tensor("b2d", [64, 1], F32, kind="ExternalInput")
    b3d = nc.dram_tensor("b3d", [32, 1], F32, kind="ExternalInput")
    b4d = nc.dram_tensor("b4d", [18, 1], F32, kind="ExternalInput")
    i96d = nc.dram_tensor("i96d", [96, 256], F32, kind="ExternalInput")
    j96d = nc.dram_tensor("j96d", [96, 256], F32, kind="ExternalInput")
    seld = nc.dram_tensor("seld", [96, 16], BF, kind="ExternalInput")

    out_part = nc.dram_tensor("out_part", [2, 128, HW], BF, kind="ExternalOutput")

    with tile.TileContext(nc) as tc:
        with tc.tile_pool(name="consts", bufs=1) as consts, \
             tc.tile_pool(name="wpool", bufs=1) as wpool, \
             tc.tile_pool(name="hbufs", bufs=1) as hbufs, \
             tc.tile_pool(name="psum", bufs=3, space="PSUM") as psum_pool, \
             tc.tile_pool(name="dram", bufs=1, space="DRAM") as dram:

            # warp images (flat + shifted, mask pre-applied, 2 channel tiles)
            _early = contextlib.ExitStack()
            imgs = _early.enter_context(tc.tile_pool(name="imgs2", bufs=1))
            imf = [imgs.tile([128, NFLAT], BF, name=f"imf{c}") for c in range(2)]
            ims = [imgs.tile([128, NFLAT], BF, name=f"ims{c}") for c in range(2)]

            # ---- load weights / constants (w1 + inputs first on sync q) ----
            _c1 = contextlib.ExitStack()
            convin = _c1.enter_context(tc.tile_pool(name="convin", bufs=1))
            w1_t = convin.tile([128, 18, 2, 128], F8)
            nc.scalar.dma_start(w1_t[:], w1t[:, :, :, :])
            x8 = convin.tile([128, 4, NPAD], F8)
            for lo, hi in ((0, 1122), (1122, 2244), (2244, 3366), (3366, NPAD)):
                nc.sync.dma_start(x8[:, :, lo:hi], inp8[:, :, lo:hi])

            w2_t = wpool.tile([128, 9, 64], F8)
            w3_t = wpool.tile([64, 9, 32], F8)
            w4_t = wpool.tile([32, 9, 32], F8)
            b1_t = consts.tile([128, 1], F32)
            b2_t = consts.tile([64, 1], F32)
            b3_t = consts.tile([32, 1], F32)
            b4_t = consts.tile([18, 1], F32)
            i96 = consts.tile([96, 256], F32)
            j96 = consts.tile([96, 256], F32)
            sel = consts.tile([96, 16], BF)
            nc.scalar.dma_start(w2_t[:], w2t[:, :, :])
            nc.scalar.dma_start(w3_t[:], w3t[:, :, :])
            nc.scalar.dma_start(w4_t[:], w4t[:, :, :])
            nc.scalar.dma_start(b1_t[:], b1d[:, :])
            nc.scalar.dma_start(b2_t[:], b2d[:, :])
            nc.scalar.dma_start(b3_t[:], b3d[:, :])
            nc.scalar.dma_start(b4_t[:], b4d[:, :])
            nc.scalar.dma_start(i96[:], i96d[:, :])
            nc.scalar.dma_start(j96[:], j96d[:, :])
            nc.scalar.dma_start(sel[:], seld[:, :])

            # warmup collective: absorbs CC stream setup cost early
            ccw_in = dram.tile([1, 8], F32)
            ccw_out = dram.tile([1, 8], F32, addr_space="Shared")
            wseed = consts.tile([1, 8], F32)
            nc.vector.memset(wseed[:], 0.0)
            nc.gpsimd.dma_start(ccw_in[:], wseed[:])
            nc.gpsimd.collective_compute(
                "AllReduce", AluOp.add,
                replica_groups=[list(range(G))],
                ins=[ccw_in.opt()], outs=[ccw_out.opt()])

            # hidden activations: fp8, padded layout, dup copy shifted by 1
            h1 = hbufs.tile([128, 3, NPAD], F8)
            h2 = hbufs.tile([64, 3, NPAD], F8)
            h3 = hbufs.tile([32, 3, NPAD], F8)
            nc.vector.memset(h1[:], 0.0)
            nc.vector.memset(h2[:], 0.0)
            nc.vector.memset(h3[:], 0.0)

            ccs = hbufs.tile([6, HW], F32)     # exp(logits)
            oa18 = hbufs.tile([18, HW], F32)   # conv4 out: logits(6) + offsets(12)

            def conv234(src, wt, cout, drain):
                """fp8 conv via DoubleRow tap pairs (0,1),(3,4),(6,7) +
                plain-fp8 singles (2,5,8). src: [P, 2, NPAD] dup tile."""
                sv = src[:].rearrange("p t (r c) -> p t r c", c=PADW)
                for r in range(8):
                    ps = psum_pool.tile([cout, 512], F32, tag="convps", name="cp")
                    first = True
                    for t in (0, 3, 6):
                        di = t // 3
                        rhs = sv[:, 0:2, r * 8 + di:r * 8 + di + 8, 0:64]
                        nc.tensor.matmul(ps[:], wt[:, t:t + 2, :], rhs,
                                         start=first, stop=False,
                                         perf_mode=PM.DoubleRow,
                                         skip_group_check=True)
                        first = False
                    # taps (2,5) via copies 0 and 2 (the <<66 dup)
                    rhs = sv[:, 0:3:2, r * 8 + 0:r * 8 + 8, 2:66]
                    nc.tensor.matmul(ps[:], wt[:, 2:6:3, :], rhs,
                                     start=False, stop=False,
                                     perf_mode=PM.DoubleRow,
                                     skip_group_check=True)
                    rhs = sv[:, 0, r * 8 + 2:r * 8 + 10, 2:66]
                    nc.tensor.matmul(ps[:], wt[:, 8, :], rhs,
                                     start=False, stop=True,
                                     skip_group_check=True)
                    drain(r, ps)

            def drain_lrelu(dst, bias):
                dv0 = dst[:, 0, :].rearrange("p (r c) -> p r c", c=PADW)
                dv1 = dst[:, 1, :].rearrange("p (r c) -> p r c", c=PADW)
                dv2 = dst[:, 2, :].rearrange("p (r c) -> p r c", c=PADW)

                def d(r, ps):
                    nc.scalar.activation(dv0[:, r * 8 + 1:r * 8 + 9, 1:65],
                                         ps[:], ActFn.Lrelu,
                                         bias=bias[:, 0:1], scale=1.0 / WS,
                                         alpha=0.1)
                    nc.scalar.activation(dv1[:, r * 8 + 1:r * 8 + 9, 0:64],
                                         ps[:], ActFn.Lrelu,
                                         bias=bias[:, 0:1], scale=1.0 / WS,
                                         alpha=0.1)
                    nc.vector.tensor_copy(dv2[:, r * 8:r * 8 + 8, 1:65],
                                          dv0[:, r * 8 + 1:r * 8 + 9, 1:65])
                return d

            # ---- conv1: DoubleRow over input-channel slice pairs ----
            xv = x8[:].rearrange("p s (r c) -> p s r c", c=PADW)
            d1 = drain_lrelu(h1, b1_t)
            for r in range(8):
                ps = psum_pool.tile([128, 512], F32, tag="convps", name="cp1")
                i_mm = 0
                for t in range(9):
                    di, dj = t // 3, t % 3
                    for j in range(2):
                        rhs = xv[:, 2 * j:2 * j + 2,
                                 r * 8 + di:r * 8 + di + 8, dj:dj + 64]
                        nc.tensor.matmul(ps[:], w1_t[:, t * 2 + j, :, :], rhs,
                                         start=(i_mm == 0), stop=(i_mm == 17),
                                         perf_mode=PM.DoubleRow,
                                         skip_group_check=True)
                        i_mm += 1
                d1(r, ps)
            _c1.close()
            for c in range(2):
                nc.scalar.dma_start(imf[c][:], img_f[c, :, :])
                nc.scalar.dma_start(ims[c][:], img_s[c, :, :])

            conv234(h1, w2_t[:], 64, drain_lrelu(h2, b2_t))
            conv234(h2, w3_t[:], 32, drain_lrelu(h3, b3_t))

            # ---- conv4: drain, then exp() of the logit rows ----
            def d4(r, ps):
                sl = slice(r * 512, (r + 1) * 512)
                nc.scalar.activation(oa18[:, sl], ps[0:18, :], ActFn.Identity,
                                     bias=b4_t[:, 0:1], scale=1.0 / WS)
                nc.scalar.activation(ccs[:, sl], oa18[0:6, sl], ActFn.Exp)
            conv234(h3, w4_t[:], 32, d4)

            _late = contextlib.ExitStack()
            maps = _late.enter_context(tc.tile_pool(name="maps", bufs=1))
            mtmp = _late.enter_context(tc.tile_pool(name="mtmp", bufs=6))
            warp = _late.enter_context(tc.tile_pool(name="warp", bufs=3))

            # ---- softmax across groups (AllReduce of exp(logits)) ----
            cc_in = dram.tile([6, HW], F32)
            cc_out = dram.tile([6, HW], F32, addr_space="Shared")
            nc.scalar.dma_start(cc_in[:, 0:2048], ccs[:, 0:2048])
            nc.scalar.dma_start(cc_in[:, 2048:HW], ccs[:, 2048:HW])
            nc.gpsimd.collective_compute(
                "AllReduce", AluOp.add,
                replica_groups=[list(range(G))],
                ins=[cc_in.opt()], outs=[cc_out.opt()])

            # offsets to DRAM for the [96,256] reshape (parallel with CC)
            oa_d = dram.tile([12, HW], F32)
            nc.sync.dma_start(oa_d[:, 0:2048], oa18[6:18, 0:2048])
            nc.sync.dma_start(oa_d[:, 2048:HW], oa18[6:18, 2048:HW])

            # ---- [96, 256] map computation ----
            ox = maps.tile([96, 256], F32)
            oy = maps.tile([96, 256], F32)
            ex96 = maps.tile([96, 256], F32)
            tot96 = maps.tile([96, 256], F32)
            oav = oa_d[:].rearrange("a (q f) -> (a q) f", f=256)
            nc.sync.dma_start(ox[:], oav[0:96, :])
            nc.sync.dma_start(oy[:], oav[96:192, :])
            ccv_in = cc_in[:].rearrange("a (q f) -> (a q) f", f=256)
            ccv_out = cc_out[:].rearrange("a (q f) -> (a q) f", f=256)
            nc.scalar.dma_start(ex96[:], ccv_in[0:96, :])
            nc.sync.dma_start(tot96[:], ccv_out[0:96, :])

            def axis_maps(off_t, coord):
                """returns w[dv] weight tiles for dv in (-1, 0, 1)."""
                t1 = mtmp.tile([96, 256], F32, tag="t", name="t4")
                nc.vector.tensor_tensor(t1[:], off_t[:], coord[:], AluOp.add)
                x = mtmp.tile([96, 256], F32, tag="t", name="t5")
                nc.vector.tensor_scalar(x[:], t1[:], 64.0 / 63.0, -0.5,
                                        AluOp.mult, AluOp.add)
                xc = mtmp.tile([96, 256], F32, tag="t", name="t6")
                nc.vector.tensor_scalar(xc[:], x[:], 0.0, 63.0,
                                        AluOp.max, AluOp.min)
                ri = mtmp.tile([96, 256], I32, tag="ti", name="t7")
                nc.vector.tensor_copy(ri[:], xc[:])
                rf = mtmp.tile([96, 256], F32, tag="t", name="t8")
                nc.vector.tensor_copy(rf[:], ri[:])
                gt = mtmp.tile([96, 256], F32, tag="t", name="t9")
                nc.vector.tensor_tensor(gt[:], rf[:], xc[:], AluOp.is_gt)
                x0 = mtmp.tile([96, 256], F32, tag="t", name="t10")
                nc.vector.tensor_tensor(x0[:], rf[:], gt[:], AluOp.subtract)
                fx = mtmp.tile([96, 256], F32, tag="t", name="t11")
                nc.vector.tensor_tensor(fx[:], xc[:], x0[:], AluOp.subtract)
                x1 = mtmp.tile([96, 256], F32, tag="t", name="t12")
                nc.vector.tensor_scalar(x1[:], x0[:], 1.0, 63.0,
                                        AluOp.add, AluOp.min)
                d0 = mtmp.tile([96, 256], F32, tag="t", name="t13")
                nc.vector.tensor_tensor(d0[:], x0[:], coord[:], AluOp.subtract)
                d1_ = mtmp.tile([96, 256], F32, tag="t", name="t14")
                nc.vector.tensor_tensor(d1_[:], x1[:], coord[:], AluOp.subtract)
                fm = mtmp.tile([96, 256], F32, tag="t", name="t15")
                nc.vector.tensor_scalar(fm[:], fx[:], -1.0, 1.0,
                                        AluOp.mult, AluOp.add)
                ws = {}
                for dv in (-1.0, 0.0, 1.0):
                    a0 = mtmp.tile([96, 256], F32, tag="t", name="t16")
                    nc.vector.scalar_tensor_tensor(a0[:], d0[:], dv, fm[:],
                                                   AluOp.is_equal, AluOp.mult)
                    a1 = mtmp.tile([96, 256], F32, tag="t", name="t17")
                    nc.vector.scalar_tensor_tensor(a1[:], d1_[:], dv, fx[:],
                                                   AluOp.is_equal, AluOp.mult)
                    wv = maps.tile([96, 256], F32, name=f"w_{coord.name}_{int(dv)}")
                    nc.vector.tensor_tensor(wv[:], a0[:], a1[:], AluOp.add)
                    ws[int(dv)] = wv
                return ws

            wxs = axis_maps(ox, j96)
            wys = axis_maps(oy, i96)

            # attn = exp / allreduce-total (after CC completes)
            at = maps.tile([96, 256], F32)
            rc = mtmp.tile([96, 256], F32, tag="t", name="t3")
            nc.vector.reciprocal_approx_fast(rc[:], tot96[:])
            nc.vector.tensor_tensor(at[:], ex96[:], rc[:], AluOp.mult)

            prod = maps.tile([96, 2304], BF)
            for yi, dyv in enumerate((-1, 0, 1)):
                ad = mtmp.tile([96, 256], F32, tag="t", name="t18")
                nc.vector.tensor_tensor(ad[:], at[:], wys[dyv][:], AluOp.mult)
                for xi, dxv in enumerate((-1, 0, 1)):
                    di = yi * 3 + xi
                    nc.vector.tensor_tensor(prod[:, di * 256:(di + 1) * 256],
                                            ad[:], wxs[dxv][:], AluOp.mult)

            # K-sum via selection matmul -> Wd [16, 2304]
            wps = psum_pool.tile([16, 2304], F32, tag="wdps", bufs=1, name="wdps")
            wd16 = maps.tile([16, 2304], BF)
            wd_d = dram.tile([16, 2304], BF)
            for c0 in range(0, 2304, 512):
                cn = min(512, 2304 - c0)
                nc.tensor.matmul(wps[:, c0:c0 + cn], sel[:], prod[:, c0:c0 + cn],
                                 start=True, stop=True)
                nc.scalar.activation(wd16[:, c0:c0 + cn], wps[:, c0:c0 + cn],
                                     ActFn.Copy)
                nc.gpsimd.dma_start(wd_d[:, c0:c0 + cn], wd16[:, c0:c0 + cn])

            # ---- warp: out[c,p] = sum_d Wd[p] * img[c, p+d] ----
            bq = [nc.sync, nc.scalar, nc.gpsimd]
            acc = [None, None]
            for di9 in range(9):
                dy, dx = di9 // 3 - 1, di9 % 3 - 1
                wdb = warp.tile([128, HW], BF, tag="wdb", bufs=2, name="t20")
                src = wd_d[0:16, di9 * 256:(di9 + 1) * 256]
                bq[di9 % 3].dma_start(wdb[:], src.partition_broadcast(128))
                for c in range(2):
                    base = GUARD + 64 * dy
                    if dx == 0:
                        img_ap = imf[c][:, base:base + HW]
                    elif dx == 1:
                        img_ap = ims[c][:, base:base + HW]
                    else:
                        img_ap = ims[c][:, base - 2:base - 2 + HW]
                    if acc[c] is None:
                        acc[c] = warp.tile([128, HW], BF, tag=f"acc{c}", bufs=2, name="t21")
                        nc.vector.tensor_tensor(acc[c][:], img_ap, wdb[:], AluOp.mult)
                    else:
                        pr = warp.tile([128, HW], BF, tag="pr", bufs=2, name="t22")
                        nc.vector.tensor_tensor(pr[:], img_ap, wdb[:], AluOp.mult)
                        nacc = warp.tile([128, HW], BF, tag=f"acc{c}", bufs=2, name="t23")
                        nc.vector.tensor_tensor(nacc[:], acc[c][:], pr[:], AluOp.add)
                        acc[c] = nacc

            nc.sync.dma_start(out_part[0, :, :], acc[0][:])
            nc.scalar.dma_start(out_part[1, :, :], acc[1][:])
            _late.close()
            _early.close()

    nc.compile()
    return nc


def _prep_inputs(gar_feat, cond_feat, mask, W1, b1, W2, b2, W3, b3, W4, b4):
    """Host-side prep: returns list of 8 per-core input dicts."""
    gar = np.asarray(gar_feat, np.float32)[0]      # [256, 64, 64]
    cond = np.asarray(cond_feat, np.float32)[0]
    maskf = np.asarray(mask, np.float32)[0]        # [G, 256]

    inp = np.concatenate([gar, cond], axis=0)      # [512, 64, 64]
    inp_pad = np.zeros((C_IN, PADW, PADW), np.float32)
    inp_pad[:, 1:-1, 1:-1] = inp
    # [128, 4, NPAD]: partition p, slice s -> channel s*128+p
    inp8 = inp_pad.reshape(4, 128, NPAD).transpose(1, 0, 2).astype(FP8)

    i_idx = (np.arange(HW, dtype=np.float32) // W).reshape(16, 256)
    j_idx = (np.arange(HW, dtype=np.float32) % W).reshape(16, 256)
    i96 = np.tile(i_idx, (6, 1)).astype(np.float32)
    j96 = np.tile(j_idx, (6, 1)).astype(np.float32)
    sel = np.zeros((96, 16), np.float32)
    sel[np.arange(96), np.arange(96) % 16] = 1.0
    sel = sel.astype(BF16)

    per_core = []
    for g in range(G):
        w1g = np.asarray(W1[g], np.float32) * WS   # [128, 512, 3, 3]
        w2g = np.asarray(W2[g], np.float32) * WS   # [64, 128, 3, 3]
        w3g = np.asarray(W3[g], np.float32) * WS   # [32, 64, 3, 3]
        w4g = (np.asarray(W4[g], np.float32) * WS)[PERM4]  # [18, 32, 3, 3]
        b4g = np.asarray(b4[g], np.float32)[PERM4]

        # w1T[p, t*2+j, s, o] = w1g[o, (2j+s)*128+p, di, dj]
        w1T = np.zeros((128, 18, 2, 128), np.float32)
        for t in range(9):
            di, dj = t // 3, t % 3
            for jj in range(2):
                for s in range(2):
                    sl = 2 * jj + s
                    w1T[:, t * 2 + jj, s, :] = w1g[:, sl * 128:(sl + 1) * 128, di, dj].T
        w2T = np.zeros((128, 9, 64), np.float32)
        w3T = np.zeros((64, 9, 32), np.float32)
        w4T = np.zeros((32, 9, 32), np.float32)
        for t in range(9):
            di, dj = t // 3, t % 3
            w2T[:, t, :] = w2g[:, :, di, dj].T
            w3T[:, t, :] = w3g[:, :, di, dj].T
            w4T[:, t, 0:18] = w4g[:, :, di, dj].T

        garm = gar * maskf[g][:, None, None]   # fold group mask into warp img
        img_flat = np.zeros((2, 128, NFLAT), np.float32)
        img_flat[:, :, GUARD:GUARD + HW] = garm.reshape(2, 128, HW)
        img_shift = np.zeros((2, 128, NFLAT), np.float32)
        img_shift[:, :, :-1] = img_flat[:, :, 1:]

        per_core.append({
            "inp8": inp8,
            "img_f": img_flat.astype(BF16),
            "img_s": img_shift.astype(BF16),
            "w1t": w1T.astype(FP8),
            "w2t": w2T.astype(FP8),
            "w3t": w3T.astype(FP8),
            "w4t": w4T.astype(FP8),
            "b1d": np.asarray(b1[g], np.float32).reshape(128, 1),
            "b2d": np.asarray(b2[g], np.float32).reshape(64, 1),
            "b3d": np.asarray(b3[g], np.float32).reshape(32, 1),
            "b4d": b4g.reshape(18, 1),
            "i96d": i96, "j96d": j96, "seld": sel,
        })
    return per_core


def _get_nc():
    if "nc" not in _CACHE:
        _CACHE["nc"] = _build()
    return _CACHE["nc"]


def run_cores(inputs, trace=False):
    nc = _get_nc()
    in_maps = _prep_inputs(**inputs)
    res = run_bass_kernel_spmd(nc, in_maps, core_ids=list(range(G)), trace=trace)
    return res


def kernel(**inputs) -> np.ndarray:
    res = run_cores(inputs, trace=False)
    out = np.zeros((C_FEAT, HW), np.float32)
    for r in res.results:
        out += r["out_part"].reshape(C_FEAT, HW).astype(np.float32)
    return out.reshape(1, C_FEAT, H, W)
